# revision 54
# baseline (speedup 1.0000x reference)
"""5G LDPC BG1 encoder (k=8000, n=16000, r=0.5, Z=384) on 8 Trainium2 cores.

Strategy: bit-pack the batch dim (2048 rows -> 128 uint16 words, one word per
SBUF partition), so every GF(2) XOR op on a [128, X] u16 tile covers the
ENTIRE batch.  GF(2) addition is bitwise XOR on the packed words.  Each
circulant base-graph entry is one XOR of a contiguous column slice: the host
stages each core's input with every info block rotated by c*48 columns (plus
the halo each entry's window needs), so a cyclic shift is a plain slice and
the SPMD program is identical on all 8 cores.  Cores split the Z=384
circulant rows of the parity 8 ways (48 rows each); the core parity pa
(needed at all rotations by C2) is computed full-width on every core in its
own rotated frame.  XOR ops are fused in pairs via custom two-window access
patterns, and emitted in DMA-chunk order so compute streams behind the input
DMA.  The device returns packed parity only; the host assembles the codeword
{u, pa, pb}, applies the rate-matching interleaver in packed u16 space, and
unpacks to f32.
"""
import numpy as np
from contextlib import ExitStack

Z = 384
KB = 22
K = 8000
N = 16000
K_LDPC = KB * Z          # 8448
PB_BLOCKS = 19           # only pb[0:7232] survives rate matching
NBPS = 4
NQ = N // NBPS           # 4000

B_TOTAL = 2048
N_CORES = 8
W = 48                   # circulant rows per core (384/8)
N_CHUNKS = 8             # input DMA chunks
OUT_COLS = 4 * Z + PB_BLOCKS * W   # 1536 + 912 = 2448

_CACHE = {}


def _base_entries(rows, cols):
    """Recover (base_row, base_col, shift) triplets from lifted index lists."""
    rows = np.asarray(rows, np.int64)
    cols = np.asarray(cols, np.int64)
    m = (rows % Z) == 0
    br = (rows[m] // Z).astype(int)
    bc = (cols[m] // Z).astype(int)
    sh = (cols[m] % Z).astype(int)
    return list(zip(br.tolist(), bc.tolist(), sh.tolist()))


def _group(entries, n_blocks, drop_bc=()):
    g = [[] for _ in range(n_blocks)]
    for br, bc, s in entries:
        if bc in drop_bc or br >= n_blocks:
            continue
        g[br].append((bc, s))
    return g


def _layout(gA, gC1):
    """Ragged staging layout: per-block window [lo, hi) and flat offsets.

    A entries read [s, s+Z) of their (rotated) block, C1 entries [s, s+W).
    Blocks feeding A go first so the pa chain can start early.
    """
    need = {}
    a_blocks = set()
    for br in range(4):
        for bc, s in gA[br]:
            need.setdefault(bc, []).append((s, s + Z))
            a_blocks.add(bc)
    for lr in range(PB_BLOCKS):
        for bc, s in gC1[lr]:
            need.setdefault(bc, []).append((s, s + W))
    a_cnt = {}
    for br in range(4):
        for bc, s in gA[br]:
            a_cnt[bc] = a_cnt.get(bc, 0) + 1
    order = sorted(need, key=lambda b: (b not in a_blocks, a_cnt.get(b, 0), b))
    span = {b: (min(x[0] for x in need[b]), max(x[1] for x in need[b]))
            for b in need}
    flat = {}
    tot = 0
    for b in order:
        lo, hi = span[b]
        flat[b] = tot - lo
        tot += hi - lo
    # chunk boundaries (block-aligned, ~equal cols, first chunk small)
    sizes = [span[b][1] - span[b][0] for b in order]
    target = sum(sizes) / N_CHUNKS
    chunks, cur, acc = [], [], 0.0
    for b, sz in zip(order, sizes):
        cur.append(b)
        acc += sz
        if acc >= target * (len(chunks) + 1) * 0.999 and len(chunks) < N_CHUNKS - 1:
            chunks.append(cur)
            cur = []
    if cur:
        chunks.append(cur)
    chunk_of = {b: ci for ci, blks in enumerate(chunks) for b in blks}
    return order, span, flat, tot, chunks, chunk_of


def _build_program(gA, gC1, gC2, layout):
    import concourse.tile as tile
    from concourse import bacc, mybir
    from concourse.ap import AP
    from concourse.alu_op_type import AluOpType

    u16 = mybir.dt.uint16
    XOR = AluOpType.bitwise_xor
    order, span, flat, tot, chunks, chunk_of = layout

    nc = bacc.Bacc("TRN2", target_bir_lowering=False, debug=False)
    u_dram = nc.dram_tensor("u", [128, tot], u16, kind="ExternalInput").ap()
    o_dram = nc.dram_tensor("out", [128, OUT_COLS], u16, kind="ExternalOutput").ap()

    with tile.TileContext(nc) as tc, ExitStack() as ctx:
        pin = ctx.enter_context(tc.tile_pool(name="pin", bufs=1))
        pw = ctx.enter_context(tc.tile_pool(name="pw", bufs=1))

        u_sb = pin.tile([128, tot], u16, tag="u")
        au = pw.tile([128, 4, Z], u16, tag="au")
        pa = pw.tile([128, 4, Z + W - 1], u16, tag="pa")
        pb = pw.tile([128, PB_BLOCKS, W], u16, tag="pb")

        def pair_ap(base_ap, d):
            """[128, 2, W] view: window at base_ap and the one d elems later."""
            w = base_ap.ap[-1][1]
            return AP(base_ap.tensor, base_ap.offset,
                      [list(base_ap.ap[0]), [d, 2], [1, w]])

        carry = {}      # (width, src-tensor id) -> unpaired acc ops

        def emit(op_list, flush=False):
            """Emit XOR ops, fusing same-kind pairs on distinct dsts.

            op kinds: ("init", dst_ap, s0_ap, s1_ap, daddr, a0, a1)
                      ("acc",  dst_ap, src_ap, daddr, saddr)
            daddr/a*/saddr are flat element offsets used to build pair strides.
            Unpaired accs carry to a later emit() of the same width/src tensor
            (accs on one dst commute); inits are never carried.
            """
            by_kind = {"init": [], "acc": []}
            for op in op_list:
                by_kind[op[0]].append(op)
            acc_groups = {}
            for op in by_kind["acc"]:
                key = (op[1].ap[-1][1], id(op[2].tensor))
                acc_groups.setdefault(key, []).append(op)
            if flush:
                for key in list(carry):
                    acc_groups.setdefault(key, [])
            groups = [("init", None, by_kind["init"])] +                      [("acc", k, g) for k, g in acc_groups.items()]
            for kind, key, ops in groups:
                # pair ops on DISTINCT dsts (same-dst fusion would write one
                # element from two lanes); same-dst accs commute so any
                # inter-op order is fine.
                didx = 4 if kind == "init" else 3    # dst-offset field
                pending = (carry.pop(key, []) if kind == "acc" else []) + ops
                leftovers = []
                while pending:
                    a = pending.pop(0)
                    b = None
                    for j, cand in enumerate(pending):
                        if cand[didx] != a[didx]:     # different dst offset
                            b = pending.pop(j)
                            break
                    if b is None:
                        leftovers.append(a)
                        continue
                    if kind == "init":
                        _, d_a, s0_a, s1_a, da, a0, a1 = a
                        _, d_b, s0_b, s1_b, db, b0, b1 = b
                        nc.vector.tensor_tensor(
                            pair_ap(d_a, db - da),
                            pair_ap(s0_a, b0 - a0),
                            pair_ap(s1_a, b1 - a1), op=XOR)
                    else:
                        _, d_a, s_a, da, sa = a
                        _, d_b, s_b, db, sb = b
                        nc.vector.tensor_tensor(
                            pair_ap(d_a, db - da),
                            pair_ap(d_a, db - da),
                            pair_ap(s_a, sb - sa), op=XOR)
                if not leftovers:
                    continue
                if kind == "acc" and not flush:
                    carry[key] = leftovers
                    continue
                for a in leftovers:
                    if kind == "init":
                        nc.vector.tensor_tensor(a[1], a[2], a[3], op=XOR)
                    else:
                        nc.vector.tensor_tensor(a[1], a[1], a[2], op=XOR)

        # ---- chunked input DMA ----
        dma_col = 0
        chunk_cols = []
        for blks in chunks:
            w = sum(span[b][1] - span[b][0] for b in blks)
            nc.sync.dma_start(u_sb[:, dma_col:dma_col + w],
                              u_dram[:, dma_col:dma_col + w])
            dma_col += w
            chunk_cols.append(dma_col)

        def usrc(bc, s, w):
            a = flat[bc] + s
            ap = u_sb[:, a:a + w]
            return ap, int(ap.offset)

        # per-dst source streams, chunk-sorted
        AU, PBL = "au", "pb"
        dsts = []
        for br in range(4):
            srcs = sorted(gA[br], key=lambda e: chunk_of[e[0]])
            dsts.append((AU, br, Z, srcs))
        for lr in range(PB_BLOCKS):
            srcs = sorted(gC1[lr], key=lambda e: chunk_of[e[0]])
            dsts.append((PBL, lr, W, srcs))

        def dst_ap(kind, idx, w):
            ap = au[:, idx, :] if kind == AU else pb[:, idx, :]
            return ap, int(ap.offset)

        state = {id(d): 0 for d in dsts}
        a_done_emitted = False
        c2_done = set()

        def ready_ops(t, width_kind):
            out = []
            for d in dsts:
                kind, idx, w, srcs = d
                if kind != width_kind:
                    continue
                pos = state[id(d)]
                while pos < len(srcs):
                    if pos == 0:
                        if len(srcs) == 1:
                            if chunk_of[srcs[0][0]] <= t:
                                sap, _ = usrc(srcs[0][0], srcs[0][1], w)
                                dap, _ = dst_ap(kind, idx, w)
                                nc.vector.tensor_copy(dap, sap)
                                pos = 1
                                continue
                            break
                        if (chunk_of[srcs[0][0]] <= t
                                and chunk_of[srcs[1][0]] <= t):
                            s0, a0 = usrc(srcs[0][0], srcs[0][1], w)
                            s1, a1 = usrc(srcs[1][0], srcs[1][1], w)
                            dap, da = dst_ap(kind, idx, w)
                            out.append(("init", dap, s0, s1, da, a0, a1))
                            pos = 2
                            continue
                        break
                    if chunk_of[srcs[pos][0]] <= t:
                        sap, sa = usrc(srcs[pos][0], srcs[pos][1], w)
                        dap, da = dst_ap(kind, idx, w)
                        out.append(("acc", dap, sap, da, sa))
                        pos += 1
                        continue
                    break
                state[id(d)] = pos
            return out

        def au_complete():
            return all(state[id(d)] >= len(d[3]) for d in dsts if d[0] == AU)

        def emit_ready_c2():
            """C2 accs for rows whose C1 accumulation is complete."""
            batch = []
            for d in dsts:
                kind, lr, w, srcs = d
                if kind != PBL or lr in c2_done or state[id(d)] < len(srcs):
                    continue
                c2_done.add(lr)
                for bc, s in gC2[lr]:
                    sap = pa[:, bc, s:s + W]
                    dap = pb[:, lr, :]
                    batch.append(("acc", dap, sap,
                                  int(dap.offset), int(sap.offset)))
            emit(batch)

        for t in range(len(chunks)):
            emit(ready_ops(t, AU))
            emit(ready_ops(t, PBL))
            if au_complete() and not a_done_emitted:
                emit([], flush=True)      # drain carried accs before chain
                a_done_emitted = True
                # pa chain + halo (copies on the idle Act engine)
                nc.scalar.copy(pa[:, 0, 0:Z], au[:, 0, :])
                for i in range(1, 4):
                    nc.vector.tensor_tensor(pa[:, i, 0:Z], pa[:, i - 1, 0:Z],
                                            au[:, i, :], op=XOR)
                nc.scalar.copy(pa[:, :, Z:Z + W - 1], pa[:, :, 0:W - 1])
                # pa is final: DMA it out now (overlaps remaining compute)
                nc.sync.dma_start(
                    o_dram[:, 0:4 * Z].rearrange("p (a b) -> p a b", b=Z),
                    pa[:, :, 0:Z])
            if a_done_emitted:
                emit_ready_c2()

        emit([], flush=True)              # drain remaining carried accs
        assert not carry
        assert a_done_emitted and au_complete()
        assert all(state[id(d)] >= len(d[3]) for d in dsts)
        assert len(c2_done) == PB_BLOCKS

        nc.scalar.dma_start(
            o_dram[:, 4 * Z:4 * Z + 14 * W].rearrange("p (a b) -> p a b", b=W),
            pb[:, 0:14, :])
        nc.sync.dma_start(
            o_dram[:, 4 * Z + 14 * W:OUT_COLS].rearrange("p (a b) -> p a b", b=W),
            pb[:, 14:PB_BLOCKS, :])

    return nc


def _get_program(a_rows, a_cols, bi_rows, bi_cols, c1_rows, c1_cols,
                 c2_rows, c2_cols):
    if "prog" in _CACHE:
        return _CACHE["prog"], _CACHE["layout"]
    entB = _base_entries(bi_rows, bi_cols)
    assert sorted(entB) == [(i, j, 0) for i in range(4) for j in range(i + 1)]
    gA = _group(_base_entries(a_rows, a_cols), 4, drop_bc=(21,))
    gC1 = _group(_base_entries(c1_rows, c1_cols), PB_BLOCKS, drop_bc=(21,))
    gC2 = _group(_base_entries(c2_rows, c2_cols), PB_BLOCKS)
    layout = _layout(gA, gC1)
    nc = _build_program(gA, gC1, gC2, layout)
    nc.compile()
    _CACHE["prog"] = nc
    _CACHE["layout"] = layout
    return nc, layout


def _pack_words(u):
    """[2048, 8000] f32 0/1 -> packed u16 [128, 8448]; bit k of word p = row k*128+p."""
    ub = (np.asarray(u) != 0).astype(np.uint8)
    ub = np.concatenate([ub, np.zeros((B_TOTAL, K_LDPC - K), np.uint8)], 1)
    w = np.ascontiguousarray(ub.reshape(16, 128, K_LDPC).transpose(1, 2, 0))
    w = np.packbits(w, axis=-1, bitorder="little")          # [128, 8448, 2]
    return np.ascontiguousarray(w).view(np.uint16)[..., 0]  # [128, 8448]


def _unpack_words(words):
    """packed u16 [128, 16000] -> f32 [2048, 16000]."""
    u8 = np.ascontiguousarray(words).view(np.uint8).reshape(128, N, 2)
    bits = np.unpackbits(u8, axis=-1, bitorder="little")    # [128, N, 16]
    return bits.transpose(2, 0, 1).reshape(B_TOTAL, N).astype(np.float32)


def kernel(u, a_rows, a_cols, bi_rows, bi_cols, c1_rows, c1_cols,
           c2_rows, c2_cols, out_int, **_ignored):
    from concourse.bass_utils import run_bass_kernel_spmd

    assert np.asarray(u).shape == (B_TOTAL, K)
    oi = np.asarray(out_int)
    expect = np.arange(N, dtype=oi.dtype).reshape(NBPS, NQ).T.ravel()
    assert np.array_equal(oi, expect), "unexpected output interleaver"

    nc, layout = _get_program(a_rows, a_cols, bi_rows, bi_cols,
                              c1_rows, c1_cols, c2_rows, c2_cols)
    order, span, flat, tot, chunks, chunk_of = layout

    w16 = _pack_words(u)                                    # [128, 8448]
    wb = w16.reshape(128, KB, Z)
    h3 = np.concatenate([wb, wb, wb], axis=2)               # [128, 22, 1152]
    in_maps = []
    for c in range(N_CORES):
        buf = np.empty((128, tot), np.uint16)
        for b in order:
            lo, hi = span[b]
            buf[:, flat[b] + lo:flat[b] + hi] = h3[:, b, c * W + lo:c * W + hi]
        in_maps.append({"u": buf})
    res = run_bass_kernel_spmd(nc, in_maps, core_ids=list(range(N_CORES)))

    pa = res.results[0]["out"][:, 0:4 * Z]                  # rotated frame r=0
    pb = np.empty((128, PB_BLOCKS, Z), np.uint16)
    for c in range(N_CORES):
        pb[:, :, c * W:(c + 1) * W] = \
            res.results[c]["out"][:, 4 * Z:OUT_COLS].reshape(128, PB_BLOCKS, W)

    cs = np.concatenate([w16[:, 2 * Z:K], pa,
                         pb.reshape(128, -1)[:, :7232]], axis=1)
    return _unpack_words(np.ascontiguousarray(cs[:, oi]))


# revision 55
# speedup vs baseline: 1.0756x; 1.0756x over previous
"""5G LDPC BG1 encoder (k=8000, n=16000, r=0.5, Z=384) on 8 Trainium2 cores.

Strategy: bit-pack the batch dim (2048 rows -> 128 uint16 words, one word per
SBUF partition), so every GF(2) XOR op on a [128, X] u16 tile covers the
ENTIRE batch.  GF(2) addition is bitwise XOR on the packed words.  Each
circulant base-graph entry is one XOR of a contiguous column slice: the host
stages each core's input with every info block rotated by c*48 columns (plus
the halo each entry's window needs), so a cyclic shift is a plain slice and
the SPMD program is identical on all 8 cores.  Cores split the Z=384
circulant rows of the parity 8 ways (48 rows each); the core parity pa
(needed at all rotations by C2) is computed full-width on every core in its
own rotated frame.  XOR ops are fused in pairs via custom two-window access
patterns, and emitted in DMA-chunk order so compute streams behind the input
DMA.  The device returns packed parity only; the host assembles the codeword
{u, pa, pb}, applies the rate-matching interleaver in packed u16 space, and
unpacks to f32.
"""
import numpy as np
from contextlib import ExitStack

Z = 384
KB = 22
K = 8000
N = 16000
K_LDPC = KB * Z          # 8448
PB_BLOCKS = 19           # only pb[0:7232] survives rate matching
NBPS = 4
NQ = N // NBPS           # 4000

B_TOTAL = 2048
N_CORES = 8
W = 48                   # circulant rows per core (384/8)
N_CHUNKS = 8             # input DMA chunks
OUT_COLS = 4 * Z + PB_BLOCKS * W   # 1536 + 912 = 2448

_CACHE = {}


def _base_entries(rows, cols):
    """Recover (base_row, base_col, shift) triplets from lifted index lists."""
    rows = np.asarray(rows, np.int64)
    cols = np.asarray(cols, np.int64)
    m = (rows % Z) == 0
    br = (rows[m] // Z).astype(int)
    bc = (cols[m] // Z).astype(int)
    sh = (cols[m] % Z).astype(int)
    return list(zip(br.tolist(), bc.tolist(), sh.tolist()))


def _group(entries, n_blocks, drop_bc=()):
    g = [[] for _ in range(n_blocks)]
    for br, bc, s in entries:
        if bc in drop_bc or br >= n_blocks:
            continue
        g[br].append((bc, s))
    return g


def _layout(gA, gC1):
    """Ragged staging layout: per-block window [lo, hi) and flat offsets.

    A entries read [s, s+Z) of their (rotated) block, C1 entries [s, s+W).
    Blocks feeding A go first so the pa chain can start early.
    """
    need = {}
    a_blocks = set()
    for br in range(4):
        for bc, s in gA[br]:
            need.setdefault(bc, []).append((s, s + Z))
            a_blocks.add(bc)
    for lr in range(PB_BLOCKS):
        for bc, s in gC1[lr]:
            need.setdefault(bc, []).append((s, s + W))
    a_cnt = {}
    for br in range(4):
        for bc, s in gA[br]:
            a_cnt[bc] = a_cnt.get(bc, 0) + 1
    order = sorted(need, key=lambda b: (b not in a_blocks, -a_cnt.get(b, 0), b))
    span = {b: (min(x[0] for x in need[b]), max(x[1] for x in need[b]))
            for b in need}
    flat = {}
    tot = 0
    for b in order:
        lo, hi = span[b]
        flat[b] = tot - lo
        tot += hi - lo
    # chunk boundaries (block-aligned, ~equal cols, first chunk small)
    sizes = [span[b][1] - span[b][0] for b in order]
    target = sum(sizes) / N_CHUNKS
    chunks, cur, acc = [], [], 0.0
    for b, sz in zip(order, sizes):
        cur.append(b)
        acc += sz
        if acc >= target * (len(chunks) + 1) * 0.999 and len(chunks) < N_CHUNKS - 1:
            chunks.append(cur)
            cur = []
    if cur:
        chunks.append(cur)
    chunk_of = {b: ci for ci, blks in enumerate(chunks) for b in blks}
    return order, span, flat, tot, chunks, chunk_of


def _build_program(gA, gC1, gC2, layout):
    import concourse.tile as tile
    from concourse import bacc, mybir
    from concourse.ap import AP
    from concourse.alu_op_type import AluOpType

    u16 = mybir.dt.uint16
    XOR = AluOpType.bitwise_xor
    order, span, flat, tot, chunks, chunk_of = layout

    nc = bacc.Bacc("TRN2", target_bir_lowering=False, debug=False)
    u_dram = nc.dram_tensor("u", [128, tot], u16, kind="ExternalInput").ap()
    o_dram = nc.dram_tensor("out", [128, OUT_COLS], u16, kind="ExternalOutput").ap()

    with tile.TileContext(nc) as tc, ExitStack() as ctx:
        pin = ctx.enter_context(tc.tile_pool(name="pin", bufs=1))
        pw = ctx.enter_context(tc.tile_pool(name="pw", bufs=1))

        u_sb = pin.tile([128, tot], u16, tag="u")
        au = pw.tile([128, 4, Z], u16, tag="au")
        pa = pw.tile([128, 4, Z + W - 1], u16, tag="pa")
        pb = pw.tile([128, PB_BLOCKS, W], u16, tag="pb")

        def pair_ap(base_ap, d):
            """[128, 2, W] view: window at base_ap and the one d elems later."""
            w = base_ap.ap[-1][1]
            return AP(base_ap.tensor, base_ap.offset,
                      [list(base_ap.ap[0]), [d, 2], [1, w]])

        carry = {}      # (width, src-tensor id) -> unpaired acc ops

        def emit(op_list, flush=False):
            """Emit XOR ops, fusing same-kind pairs on distinct dsts.

            op kinds: ("init", dst_ap, s0_ap, s1_ap, daddr, a0, a1)
                      ("acc",  dst_ap, src_ap, daddr, saddr)
            daddr/a*/saddr are flat element offsets used to build pair strides.
            Unpaired accs carry to a later emit() of the same width/src tensor
            (accs on one dst commute); inits are never carried.
            """
            by_kind = {"init": [], "acc": []}
            for op in op_list:
                by_kind[op[0]].append(op)
            acc_groups = {}
            for op in by_kind["acc"]:
                key = (op[1].ap[-1][1], id(op[2].tensor))
                acc_groups.setdefault(key, []).append(op)
            if flush:
                for key in list(carry):
                    acc_groups.setdefault(key, [])
            groups = [("init", None, by_kind["init"])] +                      [("acc", k, g) for k, g in acc_groups.items()]
            for kind, key, ops in groups:
                # pair ops on DISTINCT dsts (same-dst fusion would write one
                # element from two lanes); same-dst accs commute so any
                # inter-op order is fine.
                didx = 4 if kind == "init" else 3    # dst-offset field
                pending = (carry.pop(key, []) if kind == "acc" else []) + ops
                leftovers = []
                while pending:
                    a = pending.pop(0)
                    b = None
                    for j, cand in enumerate(pending):
                        if cand[didx] != a[didx]:     # different dst offset
                            b = pending.pop(j)
                            break
                    if b is None:
                        leftovers.append(a)
                        continue
                    if kind == "init":
                        _, d_a, s0_a, s1_a, da, a0, a1 = a
                        _, d_b, s0_b, s1_b, db, b0, b1 = b
                        nc.vector.tensor_tensor(
                            pair_ap(d_a, db - da),
                            pair_ap(s0_a, b0 - a0),
                            pair_ap(s1_a, b1 - a1), op=XOR)
                    else:
                        _, d_a, s_a, da, sa = a
                        _, d_b, s_b, db, sb = b
                        nc.vector.tensor_tensor(
                            pair_ap(d_a, db - da),
                            pair_ap(d_a, db - da),
                            pair_ap(s_a, sb - sa), op=XOR)
                if not leftovers:
                    continue
                if kind == "acc" and not flush:
                    carry[key] = leftovers
                    continue
                for a in leftovers:
                    if kind == "init":
                        nc.vector.tensor_tensor(a[1], a[2], a[3], op=XOR)
                    else:
                        nc.vector.tensor_tensor(a[1], a[1], a[2], op=XOR)

        # ---- chunked input DMA ----
        dma_col = 0
        chunk_cols = []
        for blks in chunks:
            w = sum(span[b][1] - span[b][0] for b in blks)
            nc.sync.dma_start(u_sb[:, dma_col:dma_col + w],
                              u_dram[:, dma_col:dma_col + w])
            dma_col += w
            chunk_cols.append(dma_col)

        def usrc(bc, s, w):
            a = flat[bc] + s
            ap = u_sb[:, a:a + w]
            return ap, int(ap.offset)

        # per-dst source streams, chunk-sorted
        AU, PBL = "au", "pb"
        dsts = []
        for br in range(4):
            srcs = sorted(gA[br], key=lambda e: chunk_of[e[0]])
            dsts.append((AU, br, Z, srcs))
        for lr in range(PB_BLOCKS):
            srcs = sorted(gC1[lr], key=lambda e: chunk_of[e[0]])
            dsts.append((PBL, lr, W, srcs))

        def dst_ap(kind, idx, w):
            ap = au[:, idx, :] if kind == AU else pb[:, idx, :]
            return ap, int(ap.offset)

        state = {id(d): 0 for d in dsts}
        a_done_emitted = False
        c2_done = set()

        def ready_ops(t, width_kind):
            out = []
            for d in dsts:
                kind, idx, w, srcs = d
                if kind != width_kind:
                    continue
                pos = state[id(d)]
                while pos < len(srcs):
                    if pos == 0:
                        if len(srcs) == 1:
                            if chunk_of[srcs[0][0]] <= t:
                                sap, _ = usrc(srcs[0][0], srcs[0][1], w)
                                dap, _ = dst_ap(kind, idx, w)
                                nc.vector.tensor_copy(dap, sap)
                                pos = 1
                                continue
                            break
                        if (chunk_of[srcs[0][0]] <= t
                                and chunk_of[srcs[1][0]] <= t):
                            s0, a0 = usrc(srcs[0][0], srcs[0][1], w)
                            s1, a1 = usrc(srcs[1][0], srcs[1][1], w)
                            dap, da = dst_ap(kind, idx, w)
                            out.append(("init", dap, s0, s1, da, a0, a1))
                            pos = 2
                            continue
                        break
                    if chunk_of[srcs[pos][0]] <= t:
                        sap, sa = usrc(srcs[pos][0], srcs[pos][1], w)
                        dap, da = dst_ap(kind, idx, w)
                        out.append(("acc", dap, sap, da, sa))
                        pos += 1
                        continue
                    break
                state[id(d)] = pos
            return out

        def au_complete():
            return all(state[id(d)] >= len(d[3]) for d in dsts if d[0] == AU)

        def emit_ready_c2():
            """C2 accs for rows whose C1 accumulation is complete."""
            batch = []
            for d in dsts:
                kind, lr, w, srcs = d
                if kind != PBL or lr in c2_done or state[id(d)] < len(srcs):
                    continue
                c2_done.add(lr)
                for bc, s in gC2[lr]:
                    sap = pa[:, bc, s:s + W]
                    dap = pb[:, lr, :]
                    batch.append(("acc", dap, sap,
                                  int(dap.offset), int(sap.offset)))
            emit(batch)

        for t in range(len(chunks)):
            emit(ready_ops(t, AU))
            emit(ready_ops(t, PBL))
            if au_complete() and not a_done_emitted:
                emit([], flush=True)      # drain carried accs before chain
                a_done_emitted = True
                # pa chain + halo (copies on the idle Act engine)
                nc.scalar.copy(pa[:, 0, 0:Z], au[:, 0, :])
                for i in range(1, 4):
                    nc.vector.tensor_tensor(pa[:, i, 0:Z], pa[:, i - 1, 0:Z],
                                            au[:, i, :], op=XOR)
                nc.scalar.copy(pa[:, :, Z:Z + W - 1], pa[:, :, 0:W - 1])
                # pa is final: DMA it out now (overlaps remaining compute)
                nc.sync.dma_start(
                    o_dram[:, 0:4 * Z].rearrange("p (a b) -> p a b", b=Z),
                    pa[:, :, 0:Z])
            if a_done_emitted:
                emit_ready_c2()

        emit([], flush=True)              # drain remaining carried accs
        assert not carry
        assert a_done_emitted and au_complete()
        assert all(state[id(d)] >= len(d[3]) for d in dsts)
        assert len(c2_done) == PB_BLOCKS

        nc.scalar.dma_start(
            o_dram[:, 4 * Z:4 * Z + 14 * W].rearrange("p (a b) -> p a b", b=W),
            pb[:, 0:14, :])
        nc.sync.dma_start(
            o_dram[:, 4 * Z + 14 * W:OUT_COLS].rearrange("p (a b) -> p a b", b=W),
            pb[:, 14:PB_BLOCKS, :])

    return nc


def _get_program(a_rows, a_cols, bi_rows, bi_cols, c1_rows, c1_cols,
                 c2_rows, c2_cols):
    if "prog" in _CACHE:
        return _CACHE["prog"], _CACHE["layout"]
    entB = _base_entries(bi_rows, bi_cols)
    assert sorted(entB) == [(i, j, 0) for i in range(4) for j in range(i + 1)]
    gA = _group(_base_entries(a_rows, a_cols), 4, drop_bc=(21,))
    gC1 = _group(_base_entries(c1_rows, c1_cols), PB_BLOCKS, drop_bc=(21,))
    gC2 = _group(_base_entries(c2_rows, c2_cols), PB_BLOCKS)
    layout = _layout(gA, gC1)
    nc = _build_program(gA, gC1, gC2, layout)
    nc.compile()
    _CACHE["prog"] = nc
    _CACHE["layout"] = layout
    return nc, layout


def _pack_words(u):
    """[2048, 8000] f32 0/1 -> packed u16 [128, 8448]; bit k of word p = row k*128+p."""
    ub = (np.asarray(u) != 0).astype(np.uint8)
    ub = np.concatenate([ub, np.zeros((B_TOTAL, K_LDPC - K), np.uint8)], 1)
    w = np.ascontiguousarray(ub.reshape(16, 128, K_LDPC).transpose(1, 2, 0))
    w = np.packbits(w, axis=-1, bitorder="little")          # [128, 8448, 2]
    return np.ascontiguousarray(w).view(np.uint16)[..., 0]  # [128, 8448]


def _unpack_words(words):
    """packed u16 [128, 16000] -> f32 [2048, 16000]."""
    u8 = np.ascontiguousarray(words).view(np.uint8).reshape(128, N, 2)
    bits = np.unpackbits(u8, axis=-1, bitorder="little")    # [128, N, 16]
    return bits.transpose(2, 0, 1).reshape(B_TOTAL, N).astype(np.float32)


def kernel(u, a_rows, a_cols, bi_rows, bi_cols, c1_rows, c1_cols,
           c2_rows, c2_cols, out_int, **_ignored):
    from concourse.bass_utils import run_bass_kernel_spmd

    assert np.asarray(u).shape == (B_TOTAL, K)
    oi = np.asarray(out_int)
    expect = np.arange(N, dtype=oi.dtype).reshape(NBPS, NQ).T.ravel()
    assert np.array_equal(oi, expect), "unexpected output interleaver"

    nc, layout = _get_program(a_rows, a_cols, bi_rows, bi_cols,
                              c1_rows, c1_cols, c2_rows, c2_cols)
    order, span, flat, tot, chunks, chunk_of = layout

    w16 = _pack_words(u)                                    # [128, 8448]
    wb = w16.reshape(128, KB, Z)
    h3 = np.concatenate([wb, wb, wb], axis=2)               # [128, 22, 1152]
    in_maps = []
    for c in range(N_CORES):
        buf = np.empty((128, tot), np.uint16)
        for b in order:
            lo, hi = span[b]
            buf[:, flat[b] + lo:flat[b] + hi] = h3[:, b, c * W + lo:c * W + hi]
        in_maps.append({"u": buf})
    res = run_bass_kernel_spmd(nc, in_maps, core_ids=list(range(N_CORES)))

    pa = res.results[0]["out"][:, 0:4 * Z]                  # rotated frame r=0
    pb = np.empty((128, PB_BLOCKS, Z), np.uint16)
    for c in range(N_CORES):
        pb[:, :, c * W:(c + 1) * W] = \
            res.results[c]["out"][:, 4 * Z:OUT_COLS].reshape(128, PB_BLOCKS, W)

    cs = np.concatenate([w16[:, 2 * Z:K], pa,
                         pb.reshape(128, -1)[:, :7232]], axis=1)
    return _unpack_words(np.ascontiguousarray(cs[:, oi]))


# revision 56
# speedup vs baseline: 1.0792x; 1.0033x over previous
"""5G LDPC BG1 encoder (k=8000, n=16000, r=0.5, Z=384) on 8 Trainium2 cores.

Strategy: bit-pack the batch dim (2048 rows -> 128 uint16 words, one word per
SBUF partition), so every GF(2) XOR op on a [128, X] u16 tile covers the
ENTIRE batch.  GF(2) addition is bitwise XOR on the packed words.  Each
circulant base-graph entry is one XOR of a contiguous column slice: the host
stages each core's input with every info block rotated by c*48 columns (plus
the halo each entry's window needs), so a cyclic shift is a plain slice and
the SPMD program is identical on all 8 cores.  Cores split the Z=384
circulant rows of the parity 8 ways (48 rows each); the core parity pa
(needed at all rotations by C2) is computed full-width on every core in its
own rotated frame.  XOR ops are fused in pairs via custom two-window access
patterns, and emitted in DMA-chunk order so compute streams behind the input
DMA.  The device returns packed parity only; the host assembles the codeword
{u, pa, pb}, applies the rate-matching interleaver in packed u16 space, and
unpacks to f32.
"""
import numpy as np
from contextlib import ExitStack

Z = 384
KB = 22
K = 8000
N = 16000
K_LDPC = KB * Z          # 8448
PB_BLOCKS = 19           # only pb[0:7232] survives rate matching
NBPS = 4
NQ = N // NBPS           # 4000

B_TOTAL = 2048
N_CORES = 8
W = 48                   # circulant rows per core (384/8)
N_CHUNKS = 8             # input DMA chunks
OUT_COLS = 4 * Z + PB_BLOCKS * W   # 1536 + 912 = 2448

_CACHE = {}


def _base_entries(rows, cols):
    """Recover (base_row, base_col, shift) triplets from lifted index lists."""
    rows = np.asarray(rows, np.int64)
    cols = np.asarray(cols, np.int64)
    m = (rows % Z) == 0
    br = (rows[m] // Z).astype(int)
    bc = (cols[m] // Z).astype(int)
    sh = (cols[m] % Z).astype(int)
    return list(zip(br.tolist(), bc.tolist(), sh.tolist()))


def _group(entries, n_blocks, drop_bc=()):
    g = [[] for _ in range(n_blocks)]
    for br, bc, s in entries:
        if bc in drop_bc or br >= n_blocks:
            continue
        g[br].append((bc, s))
    return g


def _layout(gA, gC1):
    """Ragged staging layout: per-block window [lo, hi) and flat offsets.

    A entries read [s, s+Z) of their (rotated) block, C1 entries [s, s+W).
    Blocks feeding A go first so the pa chain can start early.
    """
    need = {}
    a_blocks = set()
    for br in range(4):
        for bc, s in gA[br]:
            need.setdefault(bc, []).append((s, s + Z))
            a_blocks.add(bc)
    for lr in range(PB_BLOCKS):
        for bc, s in gC1[lr]:
            need.setdefault(bc, []).append((s, s + W))
    a_cnt = {}
    for br in range(4):
        for bc, s in gA[br]:
            a_cnt[bc] = a_cnt.get(bc, 0) + 1
    c_cnt = {}
    for lr in range(PB_BLOCKS):
        for bc, s in gC1[lr]:
            c_cnt[bc] = c_cnt.get(bc, 0) + 1
    order = sorted(need, key=lambda b: (b not in a_blocks, -a_cnt.get(b, 0),
                                        -c_cnt.get(b, 0), b))
    span = {b: (min(x[0] for x in need[b]), max(x[1] for x in need[b]))
            for b in need}
    flat = {}
    tot = 0
    for b in order:
        lo, hi = span[b]
        flat[b] = tot - lo
        tot += hi - lo
    # chunk boundaries (block-aligned, ~equal cols, first chunk small)
    sizes = [span[b][1] - span[b][0] for b in order]
    target = sum(sizes) / N_CHUNKS
    chunks, cur, acc = [], [], 0.0
    for b, sz in zip(order, sizes):
        cur.append(b)
        acc += sz
        if acc >= target * (len(chunks) + 1) * 0.999 and len(chunks) < N_CHUNKS - 1:
            chunks.append(cur)
            cur = []
    if cur:
        chunks.append(cur)
    chunk_of = {b: ci for ci, blks in enumerate(chunks) for b in blks}
    return order, span, flat, tot, chunks, chunk_of


def _build_program(gA, gC1, gC2, layout):
    import concourse.tile as tile
    from concourse import bacc, mybir
    from concourse.ap import AP
    from concourse.alu_op_type import AluOpType

    u16 = mybir.dt.uint16
    XOR = AluOpType.bitwise_xor
    order, span, flat, tot, chunks, chunk_of = layout

    nc = bacc.Bacc("TRN2", target_bir_lowering=False, debug=False)
    u_dram = nc.dram_tensor("u", [128, tot], u16, kind="ExternalInput").ap()
    o_dram = nc.dram_tensor("out", [128, OUT_COLS], u16, kind="ExternalOutput").ap()

    with tile.TileContext(nc) as tc, ExitStack() as ctx:
        pin = ctx.enter_context(tc.tile_pool(name="pin", bufs=1))
        pw = ctx.enter_context(tc.tile_pool(name="pw", bufs=1))

        u_sb = pin.tile([128, tot], u16, tag="u")
        au = pw.tile([128, 4, Z], u16, tag="au")
        pa = pw.tile([128, 4, Z + W - 1], u16, tag="pa")
        pb = pw.tile([128, PB_BLOCKS, W], u16, tag="pb")

        def pair_ap(base_ap, d):
            """[128, 2, W] view: window at base_ap and the one d elems later."""
            w = base_ap.ap[-1][1]
            return AP(base_ap.tensor, base_ap.offset,
                      [list(base_ap.ap[0]), [d, 2], [1, w]])

        carry = {}      # (width, src-tensor id) -> unpaired acc ops

        def emit(op_list, flush=False):
            """Emit XOR ops, fusing same-kind pairs on distinct dsts.

            op kinds: ("init", dst_ap, s0_ap, s1_ap, daddr, a0, a1)
                      ("acc",  dst_ap, src_ap, daddr, saddr)
            daddr/a*/saddr are flat element offsets used to build pair strides.
            Unpaired accs carry to a later emit() of the same width/src tensor
            (accs on one dst commute); inits are never carried.
            """
            by_kind = {"init": [], "acc": []}
            for op in op_list:
                by_kind[op[0]].append(op)
            acc_groups = {}
            for op in by_kind["acc"]:
                key = (op[1].ap[-1][1], id(op[2].tensor))
                acc_groups.setdefault(key, []).append(op)
            if flush:
                for key in list(carry):
                    acc_groups.setdefault(key, [])
            groups = [("init", None, by_kind["init"])] +                      [("acc", k, g) for k, g in acc_groups.items()]
            for kind, key, ops in groups:
                # pair ops on DISTINCT dsts (same-dst fusion would write one
                # element from two lanes); same-dst accs commute so any
                # inter-op order is fine.
                didx = 4 if kind == "init" else 3    # dst-offset field
                pending = (carry.pop(key, []) if kind == "acc" else []) + ops
                leftovers = []
                while pending:
                    a = pending.pop(0)
                    b = None
                    for j, cand in enumerate(pending):
                        if cand[didx] != a[didx]:     # different dst offset
                            b = pending.pop(j)
                            break
                    if b is None:
                        leftovers.append(a)
                        continue
                    if kind == "init":
                        _, d_a, s0_a, s1_a, da, a0, a1 = a
                        _, d_b, s0_b, s1_b, db, b0, b1 = b
                        nc.vector.tensor_tensor(
                            pair_ap(d_a, db - da),
                            pair_ap(s0_a, b0 - a0),
                            pair_ap(s1_a, b1 - a1), op=XOR)
                    else:
                        _, d_a, s_a, da, sa = a
                        _, d_b, s_b, db, sb = b
                        nc.vector.tensor_tensor(
                            pair_ap(d_a, db - da),
                            pair_ap(d_a, db - da),
                            pair_ap(s_a, sb - sa), op=XOR)
                if not leftovers:
                    continue
                if kind == "acc" and not flush:
                    carry[key] = leftovers
                    continue
                for a in leftovers:
                    if kind == "init":
                        nc.vector.tensor_tensor(a[1], a[2], a[3], op=XOR)
                    else:
                        nc.vector.tensor_tensor(a[1], a[1], a[2], op=XOR)

        # ---- chunked input DMA ----
        dma_col = 0
        chunk_cols = []
        for blks in chunks:
            w = sum(span[b][1] - span[b][0] for b in blks)
            nc.sync.dma_start(u_sb[:, dma_col:dma_col + w],
                              u_dram[:, dma_col:dma_col + w])
            dma_col += w
            chunk_cols.append(dma_col)

        def usrc(bc, s, w):
            a = flat[bc] + s
            ap = u_sb[:, a:a + w]
            return ap, int(ap.offset)

        # per-dst source streams, chunk-sorted
        AU, PBL = "au", "pb"
        dsts = []
        for br in range(4):
            srcs = sorted(gA[br], key=lambda e: chunk_of[e[0]])
            dsts.append((AU, br, Z, srcs))
        for lr in range(PB_BLOCKS):
            srcs = sorted(gC1[lr], key=lambda e: chunk_of[e[0]])
            dsts.append((PBL, lr, W, srcs))

        def dst_ap(kind, idx, w):
            ap = au[:, idx, :] if kind == AU else pb[:, idx, :]
            return ap, int(ap.offset)

        state = {id(d): 0 for d in dsts}
        a_done_emitted = False
        c2_done = set()

        def ready_ops(t, width_kind):
            out = []
            for d in dsts:
                kind, idx, w, srcs = d
                if kind != width_kind:
                    continue
                pos = state[id(d)]
                while pos < len(srcs):
                    if pos == 0:
                        if len(srcs) == 1:
                            if chunk_of[srcs[0][0]] <= t:
                                sap, _ = usrc(srcs[0][0], srcs[0][1], w)
                                dap, _ = dst_ap(kind, idx, w)
                                nc.vector.tensor_copy(dap, sap)
                                pos = 1
                                continue
                            break
                        if (chunk_of[srcs[0][0]] <= t
                                and chunk_of[srcs[1][0]] <= t):
                            s0, a0 = usrc(srcs[0][0], srcs[0][1], w)
                            s1, a1 = usrc(srcs[1][0], srcs[1][1], w)
                            dap, da = dst_ap(kind, idx, w)
                            out.append(("init", dap, s0, s1, da, a0, a1))
                            pos = 2
                            continue
                        break
                    if chunk_of[srcs[pos][0]] <= t:
                        sap, sa = usrc(srcs[pos][0], srcs[pos][1], w)
                        dap, da = dst_ap(kind, idx, w)
                        out.append(("acc", dap, sap, da, sa))
                        pos += 1
                        continue
                    break
                state[id(d)] = pos
            return out

        def au_complete():
            return all(state[id(d)] >= len(d[3]) for d in dsts if d[0] == AU)

        def emit_ready_c2():
            """C2 accs for rows whose C1 accumulation is complete."""
            batch = []
            for d in dsts:
                kind, lr, w, srcs = d
                if kind != PBL or lr in c2_done or state[id(d)] < len(srcs):
                    continue
                c2_done.add(lr)
                for bc, s in gC2[lr]:
                    sap = pa[:, bc, s:s + W]
                    dap = pb[:, lr, :]
                    batch.append(("acc", dap, sap,
                                  int(dap.offset), int(sap.offset)))
            emit(batch)

        for t in range(len(chunks)):
            emit(ready_ops(t, AU))
            emit(ready_ops(t, PBL))
            if au_complete() and not a_done_emitted:
                emit([], flush=True)      # drain carried accs before chain
                a_done_emitted = True
                # pa chain + halo (copies on the idle Act engine)
                nc.scalar.copy(pa[:, 0, 0:Z], au[:, 0, :])
                for i in range(1, 4):
                    nc.vector.tensor_tensor(pa[:, i, 0:Z], pa[:, i - 1, 0:Z],
                                            au[:, i, :], op=XOR)
                nc.scalar.copy(pa[:, :, Z:Z + W - 1], pa[:, :, 0:W - 1])
                # pa is final: DMA it out now (overlaps remaining compute)
                nc.sync.dma_start(
                    o_dram[:, 0:4 * Z].rearrange("p (a b) -> p a b", b=Z),
                    pa[:, :, 0:Z])
            if a_done_emitted:
                emit_ready_c2()

        emit([], flush=True)              # drain remaining carried accs
        assert not carry
        assert a_done_emitted and au_complete()
        assert all(state[id(d)] >= len(d[3]) for d in dsts)
        assert len(c2_done) == PB_BLOCKS

        nc.scalar.dma_start(
            o_dram[:, 4 * Z:4 * Z + 14 * W].rearrange("p (a b) -> p a b", b=W),
            pb[:, 0:14, :])
        nc.sync.dma_start(
            o_dram[:, 4 * Z + 14 * W:OUT_COLS].rearrange("p (a b) -> p a b", b=W),
            pb[:, 14:PB_BLOCKS, :])

    return nc


def _get_program(a_rows, a_cols, bi_rows, bi_cols, c1_rows, c1_cols,
                 c2_rows, c2_cols):
    if "prog" in _CACHE:
        return _CACHE["prog"], _CACHE["layout"]
    entB = _base_entries(bi_rows, bi_cols)
    assert sorted(entB) == [(i, j, 0) for i in range(4) for j in range(i + 1)]
    gA = _group(_base_entries(a_rows, a_cols), 4, drop_bc=(21,))
    gC1 = _group(_base_entries(c1_rows, c1_cols), PB_BLOCKS, drop_bc=(21,))
    gC2 = _group(_base_entries(c2_rows, c2_cols), PB_BLOCKS)
    layout = _layout(gA, gC1)
    nc = _build_program(gA, gC1, gC2, layout)
    nc.compile()
    _CACHE["prog"] = nc
    _CACHE["layout"] = layout
    return nc, layout


def _pack_words(u):
    """[2048, 8000] f32 0/1 -> packed u16 [128, 8448]; bit k of word p = row k*128+p."""
    ub = (np.asarray(u) != 0).astype(np.uint8)
    ub = np.concatenate([ub, np.zeros((B_TOTAL, K_LDPC - K), np.uint8)], 1)
    w = np.ascontiguousarray(ub.reshape(16, 128, K_LDPC).transpose(1, 2, 0))
    w = np.packbits(w, axis=-1, bitorder="little")          # [128, 8448, 2]
    return np.ascontiguousarray(w).view(np.uint16)[..., 0]  # [128, 8448]


def _unpack_words(words):
    """packed u16 [128, 16000] -> f32 [2048, 16000]."""
    u8 = np.ascontiguousarray(words).view(np.uint8).reshape(128, N, 2)
    bits = np.unpackbits(u8, axis=-1, bitorder="little")    # [128, N, 16]
    return bits.transpose(2, 0, 1).reshape(B_TOTAL, N).astype(np.float32)


def kernel(u, a_rows, a_cols, bi_rows, bi_cols, c1_rows, c1_cols,
           c2_rows, c2_cols, out_int, **_ignored):
    from concourse.bass_utils import run_bass_kernel_spmd

    assert np.asarray(u).shape == (B_TOTAL, K)
    oi = np.asarray(out_int)
    expect = np.arange(N, dtype=oi.dtype).reshape(NBPS, NQ).T.ravel()
    assert np.array_equal(oi, expect), "unexpected output interleaver"

    nc, layout = _get_program(a_rows, a_cols, bi_rows, bi_cols,
                              c1_rows, c1_cols, c2_rows, c2_cols)
    order, span, flat, tot, chunks, chunk_of = layout

    w16 = _pack_words(u)                                    # [128, 8448]
    wb = w16.reshape(128, KB, Z)
    h3 = np.concatenate([wb, wb, wb], axis=2)               # [128, 22, 1152]
    in_maps = []
    for c in range(N_CORES):
        buf = np.empty((128, tot), np.uint16)
        for b in order:
            lo, hi = span[b]
            buf[:, flat[b] + lo:flat[b] + hi] = h3[:, b, c * W + lo:c * W + hi]
        in_maps.append({"u": buf})
    res = run_bass_kernel_spmd(nc, in_maps, core_ids=list(range(N_CORES)))

    pa = res.results[0]["out"][:, 0:4 * Z]                  # rotated frame r=0
    pb = np.empty((128, PB_BLOCKS, Z), np.uint16)
    for c in range(N_CORES):
        pb[:, :, c * W:(c + 1) * W] = \
            res.results[c]["out"][:, 4 * Z:OUT_COLS].reshape(128, PB_BLOCKS, W)

    cs = np.concatenate([w16[:, 2 * Z:K], pa,
                         pb.reshape(128, -1)[:, :7232]], axis=1)
    return _unpack_words(np.ascontiguousarray(cs[:, oi]))
